# revision 21
# baseline (speedup 1.0000x reference)
"""Bass/Tile kernel for masked multi-head attention on 8 trn2 NeuronCores.

Problem (hardcoded shapes): B=4, S=2048, DM=1024, H=16, D=64.
  q = Q_seq @ WQ, k = K_seq @ WK, v = V_seq @ WV  (per-head split, D=64)
  A = softmax(q k^T / 8  masked to keys < V_len[b])
  O = (A v) masked to queries < Q_len[b]

Sharding: core c owns head pair hp=c (heads 2c, 2c+1) of EVERY batch.
All cores run an identical (SPMD) program; per-core data = W column slices.

v2 design (vs the v1 baseline):
  - Queries of all batches are PACKED host-side into contiguous 512-wide
    spans (no per-batch padding): attention work drops ~24%.
  - Keys are packed at 128 granularity across batches.
  - Projections stream through a small staging ring (per-512-col DMAs)
    instead of whole-batch staging buffers.
  - v is projected in the transposed orientation (N=512 matmuls) and
    flipped back with PE-transposes: fewer, larger matmuls.
  - Key-length masking via a per-partition bias on the Exp activation
    (exp(x + (-30000)) == 0 for masked key rows) -- no ones bookkeeping.
  - Projection and attention emission are INTERLEAVED (pump schedule):
    attention cells are emitted as soon as their projected data exists,
    so attention fills the PE while the DMA streams later activations.
  - Tensor-engine work is software-pipelined: scores(cell i+1) is
    emitted before AV(cell i) so the PE never waits on the exp.
  - Output: unnormalized O^T (bf16) + denominators (f32); host divides,
    transposes and scatters into the full [B, S, H*D] result.
"""

import math
import os

import ml_dtypes
import numpy as np

B, S, DM, H, D = 4, 2048, 1024, 16, 64
P = 128
NCORES = 8
QSPAN = 512
NEG_BIAS = -30000.0

LAST_EXEC_NS = None
LAST_RESULTS = None

_PROGRAM_CACHE = {}


def _ceil(a, b):
    return -(-a // b)


def _split_excess_waits(nc, mybir):
    """Move semaphore waits beyond each instruction's encoding limit onto
    preceding same-engine NoOps (walrus rejects >1 sync wait per op)."""
    uid = 0
    for fn in nc.m.functions:
        for blk in fn.blocks:
            insts = blk.instructions
            out = []
            changed = False
            for inst in insts:
                si = inst.sync_info
                waits = list(si.on_wait) if si is not None and si.on_wait else []
                limit = 1
                if len(waits) > limit:
                    for w in waits[:-limit] if limit else waits:
                        nop = mybir.InstNoOp(name=f"wsplit-{uid}", ins=[],
                                             outs=[])
                        uid += 1
                        nop.engine = inst.engine
                        nop.sync_info = mybir.SyncInfo(on_wait=[w],
                                                       on_update=[])
                        out.append(nop)
                    si.on_wait = waits[-limit:] if limit else []
                    changed = True
                out.append(inst)
            if changed:
                blk.instructions = out


def _geometry(qlen, vlen):
    """Packing geometry shared by program build and host assembly."""
    active = [b for b in range(B) if qlen[b] > 0 and vlen[b] > 0]
    order = sorted(active, key=lambda b: (-vlen[b], b))
    nkt = {b: _ceil(vlen[b], P) for b in active}
    koff = {}
    c = 0
    for b in order:
        koff[b] = c          # key tile offset (in tiles)
        c += nkt[b]
    NKT = c
    NK = NKT * P
    qoff = {}
    c = 0
    for b in order:
        qoff[b] = c
        c += qlen[b]
    NQR = c
    NSP = _ceil(NQR, QSPAN) if NQR else 0
    NQ = NSP * QSPAN
    NKS = _ceil(NK, 512) if NK else 0
    # span subranges: list per span of dicts (b, c0, c1, bq0)
    span_subs = [[] for _ in range(NSP)]
    for b in order:
        q0 = qoff[b]
        rem = qlen[b]
        bq = 0
        while rem > 0:
            sp, c0 = divmod(q0, QSPAN)
            w = min(QSPAN - c0, rem)
            span_subs[sp].append(dict(b=b, c0=c0, c1=c0 + w, bq0=bq))
            q0 += w
            bq += w
            rem -= w
    return dict(active=active, order=order, nkt=nkt, koff=koff, NK=NK,
                qoff=qoff, NQR=NQR, NSP=NSP, NQ=NQ, NKS=NKS,
                span_subs=span_subs)


def _build_program(qlen, vlen):
    import concourse.bass as bass
    import concourse.mybir as mybir
    import concourse.tile as tile

    bf16 = mybir.dt.bfloat16
    f32 = mybir.dt.float32
    AF = mybir.ActivationFunctionType

    g = _geometry(qlen, vlen)
    nkt, koff = g["nkt"], g["koff"]
    NK, NSP, NQ, NKS = g["NK"], g["NSP"], g["NQ"], g["NKS"]
    span_subs = g["span_subs"]
    active = g["active"]

    nc = bass.Bass(
        "TRN2",
        target_bir_lowering=False,
        debug=False,
        enable_asserts=False,
        num_devices=NCORES,
    )

    qt_d = nc.dram_tensor("qt", [P, NSP, DM // P, 512], bf16,
                          kind="ExternalInput").ap()
    kt_d = nc.dram_tensor("kt", [P, NKS, DM // P, 512], bf16,
                          kind="ExternalInput").ap()
    vt_d = nc.dram_tensor("vt", [P, NKS, DM // P, 512], bf16,
                          kind="ExternalInput").ap()
    wq_d = nc.dram_tensor("wq", [P, DM // P, P], bf16,
                          kind="ExternalInput").ap()
    wk_d = nc.dram_tensor("wk", [P, DM // P, P], bf16,
                          kind="ExternalInput").ap()
    wv_d = nc.dram_tensor("wv", [P, DM // P, P], bf16,
                          kind="ExternalInput").ap()
    idm_d = nc.dram_tensor("idm", [P, P], bf16, kind="ExternalInput").ap()
    nbias = sum(1 for b in g["active"] if vlen[b] % P)
    bias_d = (nc.dram_tensor("bias", [nbias, P, 1], f32,
                             kind="ExternalInput").ap() if nbias else None)
    ot_d = nc.dram_tensor("ot", [NSP, P, QSPAN], bf16,
                          kind="ExternalOutput").ap()
    den_d = nc.dram_tensor("den", [NSP, 33, QSPAN], f32,
                           kind="ExternalOutput").ap()

    with tile.TileContext(nc) as tc:
        with (
            tc.tile_pool(name="wpool", bufs=1) as wpool,
            tc.tile_pool(name="stage", bufs=12) as stage,
            tc.tile_pool(name="proj", bufs=1) as projpool,
            tc.tile_pool(name="vt2", bufs=2) as vtpool,
            tc.tile_pool(name="exp", bufs=3) as expool,
            tc.tile_pool(name="outp", bufs=2) as outp,
            tc.tile_pool(name="pp", bufs=2, space="PSUM") as pp,
            tc.tile_pool(name="psc", bufs=2, space="PSUM") as pscp,
            tc.tile_pool(name="ppo", bufs=1, space="PSUM") as ppo,
            tc.tile_pool(name="ppd", bufs=1, space="PSUM") as ppd,
        ):
            w_sb = {}

            def load_w(name, ap):
                t = wpool.tile([P, DM // P, P], bf16, tag=f"w_{name}",
                               name=f"w_{name}")
                nc.sync.dma_start(t, ap)
                w_sb[name] = t

            ident = wpool.tile([P, P], bf16, tag="ident")
            ones = wpool.tile([P, 64], bf16, tag="ones")
            nc.vector.memset(ones, 1.0)
            # PE warmup: ramp the clock during the initial DMA wait
            warm = wpool.tile([P, P], bf16, tag="warm")
            nc.vector.memset(warm, 1.0)
            pwarm = pp.tile([P, 512], f32, tag="pp", name="pwarm")
            for _ in range(48):
                nc.tensor.matmul(pwarm[:, 0:P], lhsT=warm, rhs=warm,
                                 start=True, stop=True)
            bias_sb = {}

            qT = projpool.tile([P, NQ], bf16, tag="qT")
            kT = projpool.tile([P, NK], bf16, tag="kT")
            vn = projpool.tile([P, NK], bf16, tag="vn")

            # ---- interleaved projection + attention (pump schedule) ----
            # Projections stream span-by-span; attention cells are emitted
            # as soon as their qT / kT / vn data is projected, so the PE
            # fills DMA-bound projection gaps with attention work.
            #
            # Attention cells are processed SUB-MAJOR within each span
            # (each sub's full kt sequence consecutively): same-tile-
            # position matmuls with disjoint psum regions issued back-to-
            # back corrupt each other on hardware, so keep the v1-proven
            # pattern where consecutive same-position matmuls are always
            # separated by the other phases.

            NQR = g["NQR"]

            def emit_qproj(s):
                w = min(QSPAN, NQR - s * QSPAN)
                st = stage.tile([P, DM // P, 512], bf16, tag="st")
                if s == 0:
                    # split the first staging DMA: the first chunk matmuls
                    # can start while the second half is still in flight
                    nc.sync.dma_start(st[:, 0:2], qt_d[:, s, 0:2])
                    nc.sync.dma_start(st[:, 2:4], qt_d[:, s, 2:4])
                    nc.sync.dma_start(st[:, 4:8], qt_d[:, s, 4:8])
                else:
                    nc.sync.dma_start(st, qt_d[:, s])
                pv = pp.tile([P, 512], f32, tag="pp")
                for ch in range(DM // P):
                    nc.tensor.matmul(pv[:, :w], lhsT=w_sb["wq"][:, ch],
                                     rhs=st[:, ch, :w], start=(ch == 0),
                                     stop=(ch == DM // P - 1))
                nc.vector.tensor_copy(qT[:, s * QSPAN:s * QSPAN + w],
                                      pv[:, :w])

            def emit_kproj(s):
                w = min(512, NK - s * 512)
                st = stage.tile([P, DM // P, 512], bf16, tag="st")
                nc.sync.dma_start(st, kt_d[:, s])
                pv = pp.tile([P, 512], f32, tag="pp")
                for ch in range(DM // P):
                    nc.tensor.matmul(pv[:, :w], lhsT=w_sb["wk"][:, ch],
                                     rhs=st[:, ch, :w], start=(ch == 0),
                                     stop=(ch == DM // P - 1))
                nc.vector.tensor_copy(kT[:, s * 512:s * 512 + w], pv[:, :w])

            def emit_vproj(s):
                # project transposed (vT), then PE-transpose back per block
                w = min(512, NK - s * 512)
                st = stage.tile([P, DM // P, 512], bf16, tag="st")
                nc.sync.dma_start(st, vt_d[:, s])
                pv = pp.tile([P, 512], f32, tag="pp")
                for ch in range(DM // P):
                    nc.tensor.matmul(pv[:, :w], lhsT=w_sb["wv"][:, ch],
                                     rhs=st[:, ch, :w], start=(ch == 0),
                                     stop=(ch == DM // P - 1))
                vtmp = vtpool.tile([P, 512], bf16, tag="vt")
                nc.vector.tensor_copy(vtmp[:, :w], pv[:, :w])
                ptr = pp.tile([P, 512], bf16, tag="pp")
                for j in range(w // P):
                    nc.tensor.transpose(ptr[:, j * P:(j + 1) * P],
                                        vtmp[:, j * P:(j + 1) * P], ident)
                nc.vector.tensor_copy(vn[:, s * 512:s * 512 + w], ptr[:, :w])

            # global attention cell list (span-major, sub-major, kt asc)
            cells = []
            for sp in range(NSP):
                for s in span_subs[sp]:
                    for kt in range(nkt[s["b"]]):
                        # pump after which this cell's data is ready
                        rdy = max(sp, (koff[s["b"]] + kt) // 4)
                        last_of_span = (s is span_subs[sp][-1]
                                        and kt == nkt[s["b"]] - 1)
                        cells.append((sp, s, kt, rdy, last_of_span))

            po_pd = {}   # sp -> (po, pd), allocated lazily

            def emit_scores(cell):
                sp, s, kt, _, _ = cell
                b = s["b"]
                ktile = (koff[b] + kt) * P
                q0 = sp * QSPAN
                psc = pscp.tile([P, 2, QSPAN], f32, tag="sc")
                for h in (0, 1):
                    nc.tensor.matmul(
                        psc[:, h, s["c0"]:s["c1"]],
                        lhsT=kT[h * 64:(h + 1) * 64, ktile:ktile + P],
                        rhs=qT[h * 64:(h + 1) * 64,
                               q0 + s["c0"]:q0 + s["c1"]],
                        start=True, stop=True,
                        tile_position=(h * 64, 0))
                ex = expool.tile([P, 2, QSPAN], bf16, tag="exp")
                bias = bias_sb.get(b) if kt == nkt[b] - 1 else None
                nc.scalar.activation(
                    ex[:, :, s["c0"]:s["c1"]],
                    psc[:, :, s["c0"]:s["c1"]], AF.Exp,
                    bias=(bias if bias is not None else 0.0))
                return ex

            def emit_avden(cell, ex):
                sp, s, kt, _, last_of_span = cell
                b = s["b"]
                if sp not in po_pd:
                    po_pd[sp] = (ppo.tile([P, QSPAN], f32, tag="po",
                                          name=f"po{sp}"),
                                 ppd.tile([P, QSPAN], f32, tag="pd",
                                          name=f"pd{sp}"))
                po, pd = po_pd[sp]
                vcol = (koff[b] + kt) * P
                first, last = kt == 0, kt == nkt[b] - 1
                for h in (0, 1):
                    nc.tensor.matmul(
                        po[h * 64:(h + 1) * 64, s["c0"]:s["c1"]],
                        lhsT=vn[:, vcol + h * 64:vcol + h * 64 + 64],
                        rhs=ex[:, h, s["c0"]:s["c1"]],
                        start=first, stop=last,
                        tile_position=(0, h * 64))
                for h in (0, 1):
                    row = 64 * h
                    nc.tensor.matmul(
                        pd[row:row + 64, s["c0"]:s["c1"]],
                        lhsT=ones,
                        rhs=ex[:, h, s["c0"]:s["c1"]],
                        start=first, stop=last,
                        tile_position=(0, row))
                if last_of_span:
                    rw = max(s2["c1"] for s2 in span_subs[sp])
                    osb = outp.tile([P, QSPAN], bf16, tag="osb")
                    nc.vector.tensor_copy(osb[:, :rw], po[:, :rw])
                    nc.sync.dma_start(ot_d[sp][:, 0:rw], osb[:, :rw])
                    dsb = outp.tile([33, QSPAN], f32, tag="dsb")
                    nc.vector.tensor_copy(dsb[0:1, :rw], pd[0:1, :rw])
                    nc.vector.tensor_copy(dsb[32:33, :rw], pd[64:65, :rw])
                    nc.sync.dma_start(den_d[sp][0:1, 0:rw], dsb[0:1, :rw])
                    nc.sync.dma_start(den_d[sp][32:33, 0:rw],
                                      dsb[32:33, :rw])

            state = {"ci": 0, "pending": None}

            def pump_cells(limit):
                # emit all cells ready by pump `limit`, software-pipelined
                # one cell ahead (scores of cell i+1 before AV/den of i)
                while (state["ci"] < len(cells)
                       and cells[state["ci"]][3] <= limit):
                    cell = cells[state["ci"]]
                    ex = emit_scores(cell)
                    if state["pending"] is not None:
                        emit_avden(*state["pending"])
                    state["pending"] = (cell, ex)
                    state["ci"] += 1

            # startup: only wq blocks the first projection; the other
            # constants stream while qproj(0) computes
            load_w("wq", wq_d)
            emit_qproj(0)
            load_w("wk", wk_d)
            load_w("wv", wv_d)
            nc.sync.dma_start(ident, idm_d)
            bi = 0
            for b in active:
                if vlen[b] % P:
                    t = wpool.tile([P, 1], f32, tag=f"bias{b}",
                                   name=f"bias{b}")
                    nc.sync.dma_start(t, bias_d[bi])
                    bias_sb[b] = t
                    bi += 1

            npumps = max(NKS, NSP)
            for ks in range(npumps):
                if 0 < ks < NSP:
                    emit_qproj(ks)
                if ks < NKS:
                    emit_kproj(ks)
                    emit_vproj(ks)
                pump_cells(ks)
            pump_cells(10 ** 9)
            if state["pending"] is not None:
                emit_avden(*state["pending"])

    _split_excess_waits(nc, mybir)
    return nc, g


def kernel(Q_seq, K_seq, V_seq, Q_len, V_len, WQ, WK, WV):
    global LAST_EXEC_NS, LAST_RESULTS
    import concourse.bass_utils as bass_utils

    Q_seq = np.ascontiguousarray(np.asarray(Q_seq, dtype=np.float32))
    K_seq = np.ascontiguousarray(np.asarray(K_seq, dtype=np.float32))
    V_seq = np.ascontiguousarray(np.asarray(V_seq, dtype=np.float32))
    WQ = np.asarray(WQ, dtype=np.float32)
    WK = np.asarray(WK, dtype=np.float32)
    WV = np.asarray(WV, dtype=np.float32)
    qlen = [min(max(int(x), 0), S) for x in np.asarray(Q_len).ravel()]
    vlen = [min(max(int(x), 0), S) for x in np.asarray(V_len).ravel()]

    bf = ml_dtypes.bfloat16
    out = np.zeros((B, S, H * D), dtype=np.float32)

    # Degenerate batches (V_len==0): reference softmax of an all-masked row
    # is uniform over all S keys -> O row = mean of v rows.
    for b in range(B):
        if vlen[b] == 0 and qlen[b] > 0:
            v = V_seq[b] @ WV
            out[b, :qlen[b], :] = v.mean(axis=0, keepdims=True)

    key = (tuple(qlen), tuple(vlen))
    if key not in _PROGRAM_CACHE:
        _PROGRAM_CACHE[key] = _build_program(qlen, vlen)
    nc, g = _PROGRAM_CACHE[key]

    active, qoff, koff = g["active"], g["qoff"], g["koff"]
    if active:
        NQ, NK, NSP = g["NQ"], g["NK"], g["NSP"]
        NKS = g["NKS"]
        NKP = NKS * 512
        qt = np.zeros((DM, NQ), dtype=bf)
        kt = np.zeros((DM, NKP), dtype=bf)
        vt = np.zeros((DM, NKP), dtype=bf)
        for b in active:
            qt[:, qoff[b]:qoff[b] + qlen[b]] = Q_seq[b, :qlen[b], :].T
            k0 = koff[b] * P
            kt[:, k0:k0 + vlen[b]] = K_seq[b, :vlen[b], :].T
            vt[:, k0:k0 + vlen[b]] = V_seq[b, :vlen[b], :].T
        # span-major staged layout [P, spans, chunks, 512]
        qt = np.ascontiguousarray(
            qt.reshape(DM // P, P, NQ // 512, 512).transpose(1, 2, 0, 3))
        kt = np.ascontiguousarray(
            kt.reshape(DM // P, P, NKS, 512).transpose(1, 2, 0, 3))
        vt = np.ascontiguousarray(
            vt.reshape(DM // P, P, NKS, 512).transpose(1, 2, 0, 3))

        WQs = (WQ / math.sqrt(D)).astype(bf)
        WKs = WK.astype(bf)
        WVs = WV.astype(bf)
        shared = dict(
            qt=qt, kt=kt, vt=vt,
            idm=np.eye(P, dtype=bf),
        )
        biases = []
        for b in active:
            rb = vlen[b] % P
            if rb:
                bv = np.zeros((P, 1), dtype=np.float32)
                bv[rb:] = NEG_BIAS
                biases.append(bv)
        if biases:
            shared["bias"] = np.stack(biases, axis=0)
        in_maps = []
        for c in range(NCORES):
            m = dict(shared)
            sl = slice(c * P, (c + 1) * P)
            for nm, W in (("wq", WQs), ("wk", WKs), ("wv", WVs)):
                m[nm] = np.ascontiguousarray(
                    W[:, sl].reshape(DM // P, P, P).transpose(1, 0, 2))
            in_maps.append(m)

        trace = bool(int(os.environ.get("KERNEL_TRACE", "0")))
        res = bass_utils.run_bass_kernel_spmd(
            nc, in_maps, core_ids=list(range(NCORES)), trace=trace)
        LAST_EXEC_NS = res.exec_time_ns
        LAST_RESULTS = res

        for c in range(NCORES):
            r = res.results[c]
            ot = r["ot"]    # [NSP, 128, 512] unnormalized O^T head pair
            den = r["den"]  # [NSP, 33, 512]; rows 0 and 32 are real
            for sp in range(NSP):
                for s in g["span_subs"][sp]:
                    b, c0, c1, bq0 = s["b"], s["c0"], s["c1"], s["bq0"]
                    w = c1 - c0
                    for h in (0, 1):
                        head = 2 * c + h
                        num = ot[sp, h * 64:(h + 1) * 64,
                                 c0:c1].astype(np.float32)
                        dd = den[sp, 32 * h, c0:c1]
                        out[b, bq0:bq0 + w, head * 64:(head + 1) * 64] = \
                            (num / dd[None, :]).T
    return out
